# revision 3
# baseline (speedup 1.0000x reference)
"""Trainium2 Bass kernel: causal multi-head attention block (dense transformer).

    qkv = x @ W_qkv.T; per-head rope(q), rope(k);
    y   = concat_h(softmax(causal(q k^T / sqrt(128))) v) @ W_proj.T + b_proj

Sharding: tensor-parallel over heads, 8 cores x 2 heads. Each core computes
its QKV shard, attention for its heads, and a partial output projection
y_i = O_i @ W_proj[:, dims_i].T in fp16; the host sums partials (+ b_proj).

Structure (tuned against loop-delta wall timing on trn2):
  - Streaming j-loop: QKV+rope for 512-token block j feeds attention and the
    output projection for block j before the next block's QKV, so the PE
    stays fed and the ACT exp tail overlaps the next block's matmuls.
  - Everything lives transposed: QT/KT are [d=128 partitions, n free]; the
    head dim is permuted to [even|odd] on the host so RoPE's pair-swap is two
    contiguous 64-partition SBUF->SBUF DMA copies.
  - Scores computed transposed per 128-key tile (ST[m, q] = K_tile^T-contract-Q)
    so attn@v consumes exp(ST) directly; exp runs in PAIRS (one ACT
    instruction per two score tiles - ACT instruction overhead ~450ns
    dominates single-tile exps).
  - Softmax denominators: DVE adds into two fp16 accumulators (2x DVE rate
    vs fp32; denominators ~1e3 so fp16 is safe), folded to fp32 and reduced
    over partitions with one ones-matmul per (head, block). The 1/rowsum
    scale is applied to the attention output (linearity) - no second pass.
  - Causal masking: 0/1 mask multiply on exp output, diagonal tiles only;
    exp never overflows fp32 here (scores ~N(0,1) after 1/sqrt(d) scale).
"""

import sys

sys.path.insert(0, "/opt/trn_rl_repo")

import numpy as np
import ml_dtypes

import concourse.bass as bass
from concourse import bacc
import concourse.mybir as mybir
import concourse.tile as tile
from concourse.bass_utils import run_bass_kernel_spmd
from concourse.masks import make_identity

N = 4096
C = 2048
H = 16
D = 128
NCORES = 8
HPC = H // NCORES
NB = N // 512
NT = N // 128
CT = C // 128
SCALE = float(D) ** -0.5
SIN_TIME = 10000.0

BF16 = mybir.dt.bfloat16
F32 = mybir.dt.float32
FP16 = mybir.dt.float16

_CACHE = {}


def _cache_tag(cfg):
    import zlib
    with open(__file__, "rb") as f:
        h = zlib.crc32(f.read())
    h = zlib.crc32(repr(sorted(cfg.items())).encode(), h)
    return 16 + (h % 4096)


CFG = dict(
    rowsum_on_pe=False,   # softmax denominator via ones-matmul (else DVE adds)
    racc="fp16",          # rowsum accumulator dtype: "fp16" or "fp32"
    exp_group=2,          # score tiles per exp instruction (1/2/3)
    pipe_depth=3,
    interleave=False,     # feed qkv(j+1)/proj(j-1) PE work into attention gaps
    shared_acc=False,     # all accumulators share mmp ring (needs interleave off)
    fill_per_pair=1,      # filler chunks popped per score-group emitted
    repeat=1,
    phase=4,              # 1=qkv only, 2=+rope/vtrans, 3=+attention, 4=full
)


class _nullpool:
    def __enter__(self):
        return None
    def __exit__(self, *a):
        return False


def build_nc(**overrides):
    cfg = dict(CFG)
    cfg.update(overrides)

    nc = bacc.Bacc(None, target_bir_lowering=False)

    xT_d = nc.dram_tensor("xT", [C, N], BF16, kind="ExternalInput")
    wqkvT_d = nc.dram_tensor("wqkvT", [C, 6 * D], BF16, kind="ExternalInput")
    wpT_d = nc.dram_tensor("wpT", [HPC * D, C], BF16, kind="ExternalInput")
    cosT_d = nc.dram_tensor("cosT", [D, N], FP16, kind="ExternalInput")
    sinT_d = nc.dram_tensor("sinT", [D, N], FP16, kind="ExternalInput")
    y_d = nc.dram_tensor("y", [N, C], FP16, kind="ExternalOutput")
    # compile-cache disambiguator (cache hashes only tensor shapes)
    nc.dram_tensor("cachetag", [_cache_tag(cfg), 1], F32, kind="ExternalInput")

    with tile.TileContext(nc) as tc:
        with (
            tc.tile_pool(name="persist", bufs=1) as persist,
            tc.tile_pool(name="xtp", bufs=2) as xtp,
            tc.tile_pool(name="etp", bufs=5) as etp,
            tc.tile_pool(name="ropep", bufs=3) as ropep,
            tc.tile_pool(name="misc", bufs=2) as misc,
            tc.tile_pool(name="ysp", bufs=2) as ysp,
            tc.tile_pool(name="mmp", bufs=2, space="PSUM") as mmp,
            tc.tile_pool(name="otp", bufs=1,
                         space="PSUM") if not cfg["shared_acc"] else _nullpool() as otp,
            tc.tile_pool(
                name="stp",
                bufs=(max(1, 6 // (2 * cfg["exp_group"])) if cfg["shared_acc"]
                      else max(1, 4 // cfg["exp_group"])),
                space="PSUM") as stp,
            tc.tile_pool(name="rsp", bufs=1,
                         space="PSUM") if not cfg["shared_acc"] else _nullpool() as rsp,
        ):
            import contextlib

            loop_ctx = (
                tc.For_i(0, cfg["repeat"], 1,
                         hint_engines=tuple(nc.engines.keys()))
                if cfg["repeat"] > 1 else contextlib.nullcontext()
            )
            with loop_ctx:
                def load_x_block(j, nchunks=1):
                    t = xtp.tile([128, CT, 512], BF16, tag="xt", name=f"xt_{j}")
                    step = CT // nchunks
                    for s in range(nchunks):
                        nc.sync.dma_start(
                            t[:, s * step:(s + 1) * step, :],
                            xT_d[s * step * 128:(s + 1) * step * 128,
                                 j * 512:(j + 1) * 512].rearrange(
                                "(t p) n -> p t n", p=128
                            ),
                        )
                    return t

                wq_s = []
                for u in range(6):
                    w = persist.tile([128, CT, 128], BF16, tag=f"wq{u}", name=f"wq{u}")
                    wq_s.append(w)

                def load_wq(u):
                    nc.sync.dma_start(
                        wq_s[u][:],
                        wqkvT_d[:, u * D:(u + 1) * D].rearrange("(t p) d -> p t d", p=128),
                    )

                cosT = persist.tile([128, N], FP16, tag="cosT", name="cosT")
                sinT = persist.tile([128, N], FP16, tag="sinT", name="sinT")
                load_wq(0)
                xt3 = load_x_block(0, nchunks=4)
                nc.sync.dma_start(cosT[:, 0:512], cosT_d[:, 0:512])
                nc.sync.dma_start(sinT[:, 0:512], sinT_d[:, 0:512])
                for u in range(1, 6):
                    load_wq(u)
                wp_s = []
                for h in range(HPC):
                    w = persist.tile([128, C], BF16, tag=f"wp{h}", name=f"wp{h}")
                    nc.sync.dma_start(w[:], wpT_d[h * D:(h + 1) * D, :])
                    wp_s.append(w)
                ones = persist.tile([128, 1], BF16, tag="ones", name="ones")
                nc.vector.memset(ones[:], 1.0)
                ones_f = persist.tile([128, 1], F32, tag="ones_f", name="ones_f")
                nc.vector.memset(ones_f[:], 1.0)
                identity = persist.tile([128, 128], BF16, tag="identity", name="identity")
                make_identity(nc, identity[:])
                masks = []
                if True:
                    for mi in range(4):
                        m = persist.tile([128, 512], BF16, tag=f"mask{mi}", name=f"mask{mi}")
                        nc.gpsimd.memset(m[:], 1.0)
                        nc.gpsimd.affine_select(
                            out=m[:], in_=m[:],
                            pattern=[[1, 512]],
                            compare_op=mybir.AluOpType.is_ge,
                            fill=0.0,
                            base=-(mi * 128),
                            channel_multiplier=-1,
                        )
                        masks.append(m)

                qk_store = []
                for u in range(4):
                    t = persist.tile([128, N], BF16, tag=f"qk{u}", name=f"qk{u}")
                    qk_store.append(t)
                v_store = []
                for h in range(HPC):
                    t = persist.tile([128, NT, 128], BF16, tag=f"v{h}", name=f"v{h}")
                    v_store.append(t)
                ots = []
                for h in range(HPC):
                    t = persist.tile([128, N], BF16, tag=f"ot{h}", name=f"ot{h}")
                    ots.append(t)

                def qkv_unit(j, xt3, u, half):
                    """Half a qkv unit: 8 matmuls; second half adds the drain."""
                    if half == 0:
                        ps = mmp.tile([128, 512], F32, tag="mm", name=f"qkvps_{j}_{u}")
                        qkv_ps[(j, u)] = ps
                        for ct in range(CT // 2):
                            nc.tensor.matmul(
                                ps[:], wq_s[u][:, ct, :], xt3[:, ct, :],
                                start=(ct == 0), stop=False,
                            )
                        return
                    ps = qkv_ps.pop((j, u))
                    for ct in range(CT // 2, CT):
                        nc.tensor.matmul(
                            ps[:], wq_s[u][:, ct, :], xt3[:, ct, :],
                            start=False, stop=(ct == CT - 1),
                        )
                    if cfg["phase"] < 2:
                        if u < 4:
                            nc.scalar.copy(qk_store[u][:, j * 512:(j + 1) * 512], ps[:])
                        else:
                            nc.scalar.copy(
                                v_store[u - 4][:, j * 4:(j + 1) * 4, :].rearrange(
                                    "p t n -> p (t n)"), ps[:])
                    elif u < 4:
                        qraw = ropep.tile([128, 512], F32, tag="qraw", name=f"qraw_{j}_{u}")
                        nc.scalar.copy(qraw[:], ps[:])
                        qswap = ropep.tile([128, 512], F32, tag="qswap", name=f"qswap_{j}_{u}")
                        nc.sync.dma_start(qswap[0:64, :], qraw[64:128, :])
                        nc.sync.dma_start(qswap[64:128, :], qraw[0:64, :])
                        dst = qk_store[u][:, j * 512:(j + 1) * 512]
                        nc.vector.tensor_mul(dst, ps[:], cosT[:, j * 512:(j + 1) * 512])
                        ut = ropep.tile([128, 512], F32, tag="ut", name=f"ut_{j}_{u}")
                        nc.gpsimd.tensor_mul(ut[:], qswap[:], sinT[:, j * 512:(j + 1) * 512])
                        nc.vector.tensor_add(dst, dst, ut[:])
                    else:
                        h = u - 4
                        vtmp = misc.tile([128, 512], BF16, tag="vtmp", name=f"vtmp_{j}_{h}")
                        nc.scalar.copy(vtmp[:], ps[:])
                        for sI in range(4):
                            pst = mmp.tile([128, 128], BF16, tag="mm", name=f"vt_{j}_{h}_{sI}")
                            nc.tensor.transpose(
                                pst[:], vtmp[:, sI * 128:(sI + 1) * 128], identity[:]
                            )
                            nc.vector.tensor_copy(
                                out=v_store[h][:, j * 4 + sI, :], in_=pst[:]
                            )

                qkv_ps = {}

                def qkv_fillers(j, xt3):
                    for u in range(6):
                        for half in range(2):
                            yield lambda u=u, half=half: qkv_unit(j, xt3, u, half)

                def proj_unit(j, nt, half, ci):
                    """One [128,512] slice of the output projection."""
                    key = (nt, half)
                    if ci == 0:
                        ys = ysp.tile([128, C // 2], FP16, tag="ys",
                                      name=f"ys_{nt}_{half}", bufs=4)
                        proj_ys[key] = ys
                    else:
                        ys = proj_ys[key]
                    cc = half * 2 + ci
                    py = mmp.tile([128, 512], F32, tag="mm", name=f"py_{nt}_{cc}")
                    for h in range(HPC):
                        nc.tensor.matmul(
                            py[:], ots[h][:, nt * 128:(nt + 1) * 128],
                            wp_s[h][:, cc * 512:(cc + 1) * 512],
                            start=(h == 0), stop=(h == HPC - 1),
                        )
                    nc.any.tensor_copy(out=ys[:, ci * 512:(ci + 1) * 512], in_=py[:])
                    if ci == 1:
                        del proj_ys[key]
                        nc.sync.dma_start(
                            y_d[nt * 128:(nt + 1) * 128,
                                half * 1024:(half + 1) * 1024], ys[:])

                proj_ys = {}

                def proj_fillers(j):
                    for nt in range(4 * j, 4 * j + 4):
                        for half in range(2):
                            for ci in range(2):
                                yield lambda nt=nt, half=half, ci=ci: proj_unit(j, nt, half, ci)

                from collections import deque
                fill_q = deque()  # entries: [label, generator]

                def pop_fill(k=1):
                    for _ in range(k):
                        if not fill_q:
                            return
                        try:
                            emit = next(fill_q[0][1])
                        except StopIteration:
                            fill_q.popleft()
                            continue
                        emit()

                def drain_fill():
                    while fill_q:
                        pop_fill()

                def drain_until(label):
                    # fully emit the generator carrying `label` (and anything
                    # queued ahead of it)
                    while any(e[0] == label for e in fill_q):
                        pop_fill()

                def attention(h, j):
                    qs = qk_store[h]
                    ks = qk_store[2 + h]
                    ntiles = 4 * j + 4
                    G = cfg["exp_group"]
                    RACC = FP16 if cfg["racc"] == "fp16" else F32
                    acc_pool = mmp if cfg["shared_acc"] else otp
                    ot_ps = acc_pool.tile([128, 512], F32, tag=("mm" if cfg["shared_acc"] else "ot"), name=f"ot_{h}_{j}")
                    if cfg["rowsum_on_pe"]:
                        rs_ps = (mmp if cfg["shared_acc"] else rsp).tile(
                            [1, 512], F32,
                            tag=("mm" if cfg["shared_acc"] else "rs"),
                            name=f"rs_{h}_{j}")
                    else:
                        Rts = []
                        for ri in range(2):
                            Rt = misc.tile([128, 512], RACC, tag=f"R{ri}",
                                           name=f"R{ri}_{h}_{j}", bufs=2)
                            nc.vector.memset(Rt[:], 0.0)
                            Rts.append(Rt)

                    def apply_mask(et_ap, t):
                        if t >= 4 * j:
                            mi = t - 4 * j
                            nc.vector.tensor_mul(et_ap, et_ap, masks[mi][:])

                    def emit_scores_group(tg):
                        ng = min(G, ntiles - tg)
                        stg = stp.tile([128, G, 512], F32, tag="st",
                                       name=f"st_{h}_{j}_{tg}")
                        for i in range(ng):
                            t = tg + i
                            nc.tensor.matmul(
                                stg[:, i, :], ks[:, t * 128:(t + 1) * 128],
                                qs[:, j * 512:(j + 1) * 512],
                                start=True, stop=True,
                            )
                        etg = etp.tile([128, G, 512], BF16, tag="et",
                                       name=f"et_{h}_{j}_{tg}")
                        nc.scalar.activation(
                            etg[:, 0:ng, :], stg[:, 0:ng, :],
                            mybir.ActivationFunctionType.Exp, scale=SCALE,
                        )
                        for i in range(ng):
                            apply_mask(etg[:, i, :], tg + i)
                        return [(tg + i, etg[:, i, :]) for i in range(ng)]

                    def emit_consume(t, et_ap):
                        if cfg["rowsum_on_pe"]:
                            nc.tensor.matmul(
                                rs_ps[:], ones[:], et_ap,
                                start=(t == 0), stop=(t == ntiles - 1),
                                skip_group_check=True,
                            )
                        else:
                            R = Rts[t % 2]
                            nc.vector.tensor_add(R[:], R[:], et_ap)
                        nc.tensor.matmul(
                            ot_ps[:], v_store[h][:, t, :], et_ap,
                            start=(t == 0), stop=(t == ntiles - 1),
                            skip_group_check=True,
                        )

                    depth = cfg["pipe_depth"]
                    pending = deque()
                    nfill = cfg["fill_per_pair"] if cfg["interleave"] else 0
                    for tg in range(0, ntiles, G):
                        pending.append(emit_scores_group(tg))
                        pop_fill(nfill)
                        if len(pending) > depth:
                            for t, ap in pending.popleft():
                                emit_consume(t, ap)
                    while pending:
                        for tt, ap in pending.popleft():
                            emit_consume(tt, ap)

                    if not cfg["rowsum_on_pe"]:
                        Rf = misc.tile([128, 512], F32, tag="Rf", name=f"Rf_{h}_{j}")
                        nc.vector.tensor_add(Rf[:], Rts[0][:], Rts[1][:])
                        rs_pool = mmp if cfg["shared_acc"] else rsp
                        rs_ps = rs_pool.tile([1, 512], F32,
                                             tag=("mm" if cfg["shared_acc"] else "rs"),
                                             name=f"rs_{h}_{j}")
                        nc.tensor.matmul(
                            rs_ps[:], ones_f[:], Rf[:], start=True, stop=True,
                        )
                    recip = misc.tile([1, 512], F32, tag="recip", name=f"recip_{h}_{j}")
                    nc.vector.reciprocal(recip[:], rs_ps[:])
                    rb = misc.tile([128, 512], F32, tag="rb", name=f"rb_{h}_{j}")
                    nc.gpsimd.partition_broadcast(rb[:], recip[:], channels=128)
                    nc.vector.tensor_mul(
                        ots[h][:, j * 512:(j + 1) * 512], ot_ps[:], rb[:]
                    )

                # ---- interleaved main loop ----
                # qkv(0) runs alone; during attention(j), fillers emit
                # proj(j-1) then qkv(j+1); leftovers drain between blocks.
                xt_cur = xt3
                xts = {0: xt_cur}
                for u in range(6):
                    for half in range(2):
                        qkv_unit(0, xt_cur, u, half)
                for j in range(NB):
                    if j + 1 < NB:
                        xts[j + 1] = load_x_block(j + 1)
                        nc.sync.dma_start(
                            cosT[:, (j + 1) * 512:(j + 2) * 512],
                            cosT_d[:, (j + 1) * 512:(j + 2) * 512])
                        nc.sync.dma_start(
                            sinT[:, (j + 1) * 512:(j + 2) * 512],
                            sinT_d[:, (j + 1) * 512:(j + 2) * 512])
                        fill_q.append((("qkv", j + 1), qkv_fillers(j + 1, xts[j + 1])))
                    if cfg["phase"] >= 3:
                        drain_until(("qkv", j))
                        for h in range(HPC):
                            attention(h, j)
                        if not cfg["interleave"]:
                            drain_fill()
                            if cfg["phase"] >= 4:
                                for f in proj_fillers(j):
                                    f()
                        elif cfg["phase"] >= 4:
                            fill_q.append((("proj", j), proj_fillers(j)))
                            if j == NB - 1:
                                drain_fill()
                    else:
                        drain_fill()
                drain_fill()

                if cfg["phase"] < 4:
                    dummy = misc.tile([128, 16], FP16, tag="dummy", name="dummy")
                    nc.vector.memset(dummy[:], 0.0)
                    nc.sync.dma_start(y_d[0:128, 0:16], dummy[:])

    nc.finalize()
    return nc


def _rope_tables():
    i = np.arange(D)
    denom = np.power(SIN_TIME, 2 * (i // 2) / D)
    pe = np.arange(N)[:, None] / denom[None, :]
    sin = np.sin(pe[:, 0::2])
    cos = np.cos(pe[:, 1::2])
    sin_pos = np.repeat(sin, 2, axis=1)
    cos_pos = np.repeat(cos, 2, axis=1)
    sin_signed = sin_pos.copy()
    sin_signed[:, 0::2] *= -1.0
    perm = np.concatenate([np.arange(0, D, 2), np.arange(1, D, 2)])
    cosT = np.ascontiguousarray(cos_pos.T[perm, :]).astype(np.float16)
    sinT = np.ascontiguousarray(sin_signed.T[perm, :]).astype(np.float16)
    return cosT, sinT, perm


def prep_in_maps(x, W_qkv, W_proj):
    bf = ml_dtypes.bfloat16
    cosT, sinT, perm = _rope_tables()
    xT = np.ascontiguousarray(x.T).astype(bf)
    WpT = W_proj.T
    in_maps = []
    for c in range(NCORES):
        h0, h1 = HPC * c, HPC * c + 1
        blocks = []
        for sec in (0, 1):
            for h in (h0, h1):
                blk = W_qkv[sec * C + h * D: sec * C + (h + 1) * D, :]
                blocks.append(blk[perm, :])
        for h in (h0, h1):
            blocks.append(W_qkv[2 * C + h * D: 2 * C + (h + 1) * D, :])
        shard = np.concatenate(blocks, axis=0)
        wqkvT = np.ascontiguousarray(shard.T).astype(bf)
        wpT = np.ascontiguousarray(WpT[h0 * D:(h1 + 1) * D, :]).astype(bf)
        in_maps.append(
            {"xT": xT, "wqkvT": wqkvT, "wpT": wpT, "cosT": cosT, "sinT": sinT}
        )
    return in_maps


def add_cachetag(in_maps, cfg=None):
    tag = _cache_tag(dict(CFG, **(cfg or {})))
    for m in in_maps:
        m["cachetag"] = np.zeros((tag, 1), np.float32)
    return in_maps


def kernel(x, W_qkv, W_proj, b_proj):
    x = np.asarray(x, dtype=np.float32)
    W_qkv = np.asarray(W_qkv, dtype=np.float32)
    W_proj = np.asarray(W_proj, dtype=np.float32)
    b_proj = np.asarray(b_proj, dtype=np.float32)

    if "nc" not in _CACHE:
        _CACHE["nc"] = build_nc()
    nc = _CACHE["nc"]
    in_maps = add_cachetag(prep_in_maps(x, W_qkv, W_proj))
    res = run_bass_kernel_spmd(nc, in_maps, core_ids=list(range(NCORES)))
    parts = np.stack([res.results[i]["y"] for i in range(NCORES)], axis=0)
    y = parts.sum(axis=0, dtype=np.float32)
    return y + b_proj[None, :]


# revision 4
# speedup vs baseline: 1.1448x; 1.1448x over previous
"""Dev variant: streaming per-block pipeline (QKV(j) -> attention(j) -> proj(j)).

See kernel.py for the algorithm docs. Differences:
  - Single j-loop: per block, QKV projection + rope feed straight into that
    block's attention and output projection, so the PE never drains waiting
    for the ACT exp tail (next block's QKV matmuls are independent).
  - Both heads' score streams are interleaved in ONE deep pipeline
    (pipe_depth=7 tile-groups in flight) so each head's exp/rowsum tail
    hides under the other head's matmuls; measured -110us vs per-head loops.
  - y output in fp16 (partials ~N(0,0.35); fp16 quantization negligible).
  - rowsum on DVE adds by default (frees 288 PE ones-matmuls).
  - final 1/rowsum scale on Pool (keeps DVE hot path short).
"""

import sys

sys.path.insert(0, "/opt/trn_rl_repo")

import numpy as np
import ml_dtypes

import concourse.bass as bass
from concourse import bacc
import concourse.mybir as mybir
import concourse.tile as tile
from concourse.bass_utils import run_bass_kernel_spmd
from concourse.masks import make_identity

N = 4096
C = 2048
H = 16
D = 128
NCORES = 8
HPC = H // NCORES
NB = N // 512
NT = N // 128
CT = C // 128
SCALE = float(D) ** -0.5
SIN_TIME = 10000.0

BF16 = mybir.dt.bfloat16
F32 = mybir.dt.float32
FP16 = mybir.dt.float16

_CACHE = {}


def _cache_tag(cfg):
    import zlib
    with open(__file__, "rb") as f:
        h = zlib.crc32(f.read())
    h = zlib.crc32(repr(sorted(cfg.items())).encode(), h)
    return 16 + (h % 4096)


CFG = dict(
    rowsum_on_pe=False,   # softmax denominator via ones-matmul (else DVE adds)
    racc="fp16",          # rowsum accumulator dtype: "fp16" or "fp32"
    exp_group=2,          # score tiles per exp instruction (1/2/3)
    pipe_depth=7,
    etp_bufs=7,
    interleave=False,     # feed qkv(j+1)/proj(j-1) PE work into attention gaps
    dual_head=True,       # interleave both heads' score streams in one pipeline
    qswap_psum=False,     # rope pair-swap DMA reads PSUM directly (no ACT copy)
    shared_acc=False,     # all accumulators share mmp ring (needs interleave off)
    fill_per_pair=1,      # filler chunks popped per score-group emitted
    repeat=1,
    phase=4,              # 1=qkv only, 2=+rope/vtrans, 3=+attention, 4=full
)


class _nullpool:
    def __enter__(self):
        return None
    def __exit__(self, *a):
        return False


def build_nc(**overrides):
    cfg = dict(CFG)
    cfg.update(overrides)

    nc = bacc.Bacc(None, target_bir_lowering=False)

    xT_d = nc.dram_tensor("xT", [C, N], BF16, kind="ExternalInput")
    wqkvT_d = nc.dram_tensor("wqkvT", [C, 6 * D], BF16, kind="ExternalInput")
    wpT_d = nc.dram_tensor("wpT", [HPC * D, C], BF16, kind="ExternalInput")
    cosT_d = nc.dram_tensor("cosT", [D, N], FP16, kind="ExternalInput")
    sinT_d = nc.dram_tensor("sinT", [D, N], FP16, kind="ExternalInput")
    y_d = nc.dram_tensor("y", [N, C], FP16, kind="ExternalOutput")
    # compile-cache disambiguator (cache hashes only tensor shapes)
    nc.dram_tensor("cachetag", [_cache_tag(cfg), 1], F32, kind="ExternalInput")

    with tile.TileContext(nc) as tc:
        with (
            tc.tile_pool(name="persist", bufs=1) as persist,
            tc.tile_pool(name="xtp", bufs=2) as xtp,
            tc.tile_pool(name="etp", bufs=cfg["etp_bufs"]) as etp,
            tc.tile_pool(name="ropep", bufs=3) as ropep,
            tc.tile_pool(name="misc", bufs=2) as misc,
            tc.tile_pool(name="ysp", bufs=2) as ysp,
            tc.tile_pool(name="mmp", bufs=2, space="PSUM") as mmp,
            tc.tile_pool(name="otp", bufs=(2 if cfg["dual_head"] else 1),
                         space="PSUM") if not cfg["shared_acc"] else _nullpool() as otp,
            tc.tile_pool(
                name="stp",
                bufs=(max(1, 6 // (2 * cfg["exp_group"])) if cfg["shared_acc"]
                      else max(1, 4 // cfg["exp_group"])),
                space="PSUM") as stp,
            tc.tile_pool(name="rsp", bufs=1, space="PSUM")
            if not (cfg["shared_acc"] or cfg["dual_head"]) else _nullpool() as rsp,
        ):
            import contextlib

            loop_ctx = (
                tc.For_i(0, cfg["repeat"], 1,
                         hint_engines=tuple(nc.engines.keys()))
                if cfg["repeat"] > 1 else contextlib.nullcontext()
            )
            with loop_ctx:
                def load_x_block(j, nchunks=1):
                    t = xtp.tile([128, CT, 512], BF16, tag="xt", name=f"xt_{j}")
                    step = CT // nchunks
                    for s in range(nchunks):
                        nc.sync.dma_start(
                            t[:, s * step:(s + 1) * step, :],
                            xT_d[s * step * 128:(s + 1) * step * 128,
                                 j * 512:(j + 1) * 512].rearrange(
                                "(t p) n -> p t n", p=128
                            ),
                        )
                    return t

                wq_s = []
                for u in range(6):
                    w = persist.tile([128, CT, 128], BF16, tag=f"wq{u}", name=f"wq{u}")
                    wq_s.append(w)

                def load_wq(u):
                    nc.sync.dma_start(
                        wq_s[u][:],
                        wqkvT_d[:, u * D:(u + 1) * D].rearrange("(t p) d -> p t d", p=128),
                    )

                cosT = persist.tile([128, N], FP16, tag="cosT", name="cosT")
                sinT = persist.tile([128, N], FP16, tag="sinT", name="sinT")
                load_wq(0)
                xt3 = load_x_block(0, nchunks=4)
                nc.sync.dma_start(cosT[:, 0:512], cosT_d[:, 0:512])
                nc.sync.dma_start(sinT[:, 0:512], sinT_d[:, 0:512])
                for u in range(1, 6):
                    load_wq(u)
                wp_s = []
                for h in range(HPC):
                    w = persist.tile([128, C], BF16, tag=f"wp{h}", name=f"wp{h}")
                    nc.sync.dma_start(w[:], wpT_d[h * D:(h + 1) * D, :])
                    wp_s.append(w)
                ones = persist.tile([128, 1], BF16, tag="ones", name="ones")
                nc.vector.memset(ones[:], 1.0)
                ones_f = persist.tile([128, 1], F32, tag="ones_f", name="ones_f")
                nc.vector.memset(ones_f[:], 1.0)
                identity = persist.tile([128, 128], BF16, tag="identity", name="identity")
                make_identity(nc, identity[:])
                masks = []
                if True:
                    for mi in range(4):
                        m = persist.tile([128, 512], BF16, tag=f"mask{mi}", name=f"mask{mi}")
                        nc.gpsimd.memset(m[:], 1.0)
                        nc.gpsimd.affine_select(
                            out=m[:], in_=m[:],
                            pattern=[[1, 512]],
                            compare_op=mybir.AluOpType.is_ge,
                            fill=0.0,
                            base=-(mi * 128),
                            channel_multiplier=-1,
                        )
                        masks.append(m)

                qk_store = []
                for u in range(4):
                    t = persist.tile([128, N], BF16, tag=f"qk{u}", name=f"qk{u}")
                    qk_store.append(t)
                v_store = []
                for h in range(HPC):
                    t = persist.tile([128, NT, 128], BF16, tag=f"v{h}", name=f"v{h}")
                    v_store.append(t)
                ots = []
                for h in range(HPC):
                    t = persist.tile([128, N], BF16, tag=f"ot{h}", name=f"ot{h}")
                    ots.append(t)

                def qkv_unit(j, xt3, u, half):
                    """Half a qkv unit: 8 matmuls; second half adds the drain."""
                    if half == 0:
                        ps = mmp.tile([128, 512], F32, tag="mm", name=f"qkvps_{j}_{u}")
                        qkv_ps[(j, u)] = ps
                        for ct in range(CT // 2):
                            nc.tensor.matmul(
                                ps[:], wq_s[u][:, ct, :], xt3[:, ct, :],
                                start=(ct == 0), stop=False,
                            )
                        return
                    ps = qkv_ps.pop((j, u))
                    for ct in range(CT // 2, CT):
                        nc.tensor.matmul(
                            ps[:], wq_s[u][:, ct, :], xt3[:, ct, :],
                            start=False, stop=(ct == CT - 1),
                        )
                    if cfg["phase"] < 2:
                        if u < 4:
                            nc.scalar.copy(qk_store[u][:, j * 512:(j + 1) * 512], ps[:])
                        else:
                            nc.scalar.copy(
                                v_store[u - 4][:, j * 4:(j + 1) * 4, :].rearrange(
                                    "p t n -> p (t n)"), ps[:])
                    elif u < 4:
                        qswap = ropep.tile([128, 512], F32, tag="qswap", name=f"qswap_{j}_{u}")
                        if cfg["qswap_psum"]:
                            nc.sync.dma_start(qswap[0:64, :], ps[64:128, :])
                            nc.sync.dma_start(qswap[64:128, :], ps[0:64, :])
                        else:
                            qraw = ropep.tile([128, 512], F32, tag="qraw", name=f"qraw_{j}_{u}")
                            nc.scalar.copy(qraw[:], ps[:])
                            nc.sync.dma_start(qswap[0:64, :], qraw[64:128, :])
                            nc.sync.dma_start(qswap[64:128, :], qraw[0:64, :])
                        dst = qk_store[u][:, j * 512:(j + 1) * 512]
                        nc.vector.tensor_mul(dst, ps[:], cosT[:, j * 512:(j + 1) * 512])
                        ut = ropep.tile([128, 512], F32, tag="ut", name=f"ut_{j}_{u}")
                        nc.gpsimd.tensor_mul(ut[:], qswap[:], sinT[:, j * 512:(j + 1) * 512])
                        nc.vector.tensor_add(dst, dst, ut[:])
                    else:
                        h = u - 4
                        vtmp = misc.tile([128, 512], BF16, tag="vtmp", name=f"vtmp_{j}_{h}")
                        nc.scalar.copy(vtmp[:], ps[:])
                        for sI in range(4):
                            pst = mmp.tile([128, 128], BF16, tag="mm", name=f"vt_{j}_{h}_{sI}")
                            nc.tensor.transpose(
                                pst[:], vtmp[:, sI * 128:(sI + 1) * 128], identity[:]
                            )
                            nc.vector.tensor_copy(
                                out=v_store[h][:, j * 4 + sI, :], in_=pst[:]
                            )

                qkv_ps = {}

                def qkv_fillers(j, xt3):
                    for u in range(6):
                        for half in range(2):
                            yield lambda u=u, half=half: qkv_unit(j, xt3, u, half)

                def proj_unit(j, nt, half, ci):
                    """One [128,512] slice of the output projection."""
                    key = (nt, half)
                    if ci == 0:
                        ys = ysp.tile([128, C // 2], FP16, tag="ys",
                                      name=f"ys_{nt}_{half}", bufs=4)
                        proj_ys[key] = ys
                    else:
                        ys = proj_ys[key]
                    cc = half * 2 + ci
                    py = mmp.tile([128, 512], F32, tag="mm", name=f"py_{nt}_{cc}")
                    for h in range(HPC):
                        nc.tensor.matmul(
                            py[:], ots[h][:, nt * 128:(nt + 1) * 128],
                            wp_s[h][:, cc * 512:(cc + 1) * 512],
                            start=(h == 0), stop=(h == HPC - 1),
                        )
                    nc.any.tensor_copy(out=ys[:, ci * 512:(ci + 1) * 512], in_=py[:])
                    if ci == 1:
                        del proj_ys[key]
                        nc.sync.dma_start(
                            y_d[nt * 128:(nt + 1) * 128,
                                half * 1024:(half + 1) * 1024], ys[:])

                proj_ys = {}

                def proj_fillers(j):
                    for nt in range(4 * j, 4 * j + 4):
                        for half in range(2):
                            for ci in range(2):
                                yield lambda nt=nt, half=half, ci=ci: proj_unit(j, nt, half, ci)

                from collections import deque
                fill_q = deque()  # entries: [label, generator]

                def pop_fill(k=1):
                    for _ in range(k):
                        if not fill_q:
                            return
                        try:
                            emit = next(fill_q[0][1])
                        except StopIteration:
                            fill_q.popleft()
                            continue
                        emit()

                def drain_fill():
                    while fill_q:
                        pop_fill()

                def drain_until(label):
                    # fully emit the generator carrying `label` (and anything
                    # queued ahead of it)
                    while any(e[0] == label for e in fill_q):
                        pop_fill()

                def attention(h, j):
                    qs = qk_store[h]
                    ks = qk_store[2 + h]
                    ntiles = 4 * j + 4
                    G = cfg["exp_group"]
                    RACC = FP16 if cfg["racc"] == "fp16" else F32
                    acc_pool = mmp if cfg["shared_acc"] else otp
                    ot_ps = acc_pool.tile([128, 512], F32, tag=("mm" if cfg["shared_acc"] else "ot"), name=f"ot_{h}_{j}")
                    if cfg["rowsum_on_pe"]:
                        rs_ps = (mmp if cfg["shared_acc"] else rsp).tile(
                            [1, 512], F32,
                            tag=("mm" if cfg["shared_acc"] else "rs"),
                            name=f"rs_{h}_{j}")
                    else:
                        Rts = []
                        for ri in range(2):
                            Rt = misc.tile([128, 512], RACC, tag=f"R{ri}",
                                           name=f"R{ri}_{h}_{j}", bufs=2)
                            nc.vector.memset(Rt[:], 0.0)
                            Rts.append(Rt)

                    def apply_mask(et_ap, t):
                        if t >= 4 * j:
                            mi = t - 4 * j
                            nc.vector.tensor_mul(et_ap, et_ap, masks[mi][:])

                    def emit_scores_group(tg):
                        ng = min(G, ntiles - tg)
                        stg = stp.tile([128, G, 512], F32, tag="st",
                                       name=f"st_{h}_{j}_{tg}")
                        for i in range(ng):
                            t = tg + i
                            nc.tensor.matmul(
                                stg[:, i, :], ks[:, t * 128:(t + 1) * 128],
                                qs[:, j * 512:(j + 1) * 512],
                                start=True, stop=True,
                            )
                        etg = etp.tile([128, G, 512], BF16, tag="et",
                                       name=f"et_{h}_{j}_{tg}")
                        nc.scalar.activation(
                            etg[:, 0:ng, :], stg[:, 0:ng, :],
                            mybir.ActivationFunctionType.Exp, scale=SCALE,
                        )
                        for i in range(ng):
                            apply_mask(etg[:, i, :], tg + i)
                        return [(tg + i, etg[:, i, :]) for i in range(ng)]

                    def emit_consume(t, et_ap):
                        if cfg["rowsum_on_pe"]:
                            nc.tensor.matmul(
                                rs_ps[:], ones[:], et_ap,
                                start=(t == 0), stop=(t == ntiles - 1),
                                skip_group_check=True,
                            )
                        else:
                            R = Rts[t % 2]
                            nc.vector.tensor_add(R[:], R[:], et_ap)
                        nc.tensor.matmul(
                            ot_ps[:], v_store[h][:, t, :], et_ap,
                            start=(t == 0), stop=(t == ntiles - 1),
                            skip_group_check=True,
                        )

                    depth = cfg["pipe_depth"]
                    pending = deque()
                    nfill = cfg["fill_per_pair"] if cfg["interleave"] else 0
                    for tg in range(0, ntiles, G):
                        pending.append(emit_scores_group(tg))
                        pop_fill(nfill)
                        if len(pending) > depth:
                            for t, ap in pending.popleft():
                                emit_consume(t, ap)
                    while pending:
                        for tt, ap in pending.popleft():
                            emit_consume(tt, ap)

                    if not cfg["rowsum_on_pe"]:
                        Rf = misc.tile([128, 512], F32, tag="Rf", name=f"Rf_{h}_{j}")
                        nc.vector.tensor_add(Rf[:], Rts[0][:], Rts[1][:])
                        rs_pool = mmp if cfg["shared_acc"] else rsp
                        rs_ps = rs_pool.tile([1, 512], F32,
                                             tag=("mm" if cfg["shared_acc"] else "rs"),
                                             name=f"rs_{h}_{j}")
                        nc.tensor.matmul(
                            rs_ps[:], ones_f[:], Rf[:], start=True, stop=True,
                        )
                    recip = misc.tile([1, 512], F32, tag="recip", name=f"recip_{h}_{j}")
                    nc.vector.reciprocal(recip[:], rs_ps[:])
                    rb = misc.tile([128, 512], F32, tag="rb", name=f"rb_{h}_{j}")
                    nc.gpsimd.partition_broadcast(rb[:], recip[:], channels=128)
                    nc.vector.tensor_mul(
                        ots[h][:, j * 512:(j + 1) * 512], ot_ps[:], rb[:]
                    )

                def attention2(j):
                    """Both heads' score streams interleaved in one pipeline so
                    each head's exp/drain tail hides under the other's work."""
                    ntiles = 4 * j + 4
                    G = cfg["exp_group"]
                    RACC = FP16 if cfg["racc"] == "fp16" else F32
                    st_h = {}
                    for h in range(HPC):
                        Rts = []
                        for ri in range(2):
                            Rt = misc.tile([128, 512], RACC, tag=f"R{ri}",
                                           name=f"R{ri}_{h}_{j}", bufs=2)
                            nc.vector.memset(Rt[:], 0.0)
                            Rts.append(Rt)
                        st_h[h] = dict(
                            ot=otp.tile([128, 512], F32, tag="ot", name=f"ot_{h}_{j}"),
                            R=Rts,
                            qs=qk_store[h], ks=qk_store[2 + h],
                        )

                    def emit_scores_group(h, tg):
                        ng = min(G, ntiles - tg)
                        stg = stp.tile([128, G, 512], F32, tag="st",
                                       name=f"st_{h}_{j}_{tg}")
                        for i in range(ng):
                            t = tg + i
                            nc.tensor.matmul(
                                stg[:, i, :],
                                st_h[h]["ks"][:, t * 128:(t + 1) * 128],
                                st_h[h]["qs"][:, j * 512:(j + 1) * 512],
                                start=True, stop=True,
                            )
                        etg = etp.tile([128, G, 512], BF16, tag="et",
                                       name=f"et_{h}_{j}_{tg}")
                        nc.scalar.activation(
                            etg[:, 0:ng, :], stg[:, 0:ng, :],
                            mybir.ActivationFunctionType.Exp, scale=SCALE,
                        )
                        for i in range(ng):
                            t = tg + i
                            if t >= 4 * j:
                                nc.vector.tensor_mul(
                                    etg[:, i, :], etg[:, i, :], masks[t - 4 * j][:])
                        return [(tg + i, etg[:, i, :]) for i in range(ng)]

                    def emit_consume(h, t, et_ap):
                        nc.vector.tensor_add(
                            st_h[h]["R"][t % 2][:], st_h[h]["R"][t % 2][:], et_ap)
                        nc.tensor.matmul(
                            st_h[h]["ot"][:], v_store[h][:, t, :], et_ap,
                            start=(t == 0), stop=(t == ntiles - 1),
                            skip_group_check=True,
                        )

                    depth = cfg["pipe_depth"]
                    pending = deque()
                    for tg in range(0, ntiles, G):
                        for h in range(HPC):
                            pending.append((h, emit_scores_group(h, tg)))
                            if len(pending) > depth:
                                hh, items = pending.popleft()
                                for t, ap in items:
                                    emit_consume(hh, t, ap)
                    while pending:
                        hh, items = pending.popleft()
                        for t, ap in items:
                            emit_consume(hh, t, ap)

                    for h in range(HPC):
                        Rts = st_h[h]["R"]
                        Rf = misc.tile([128, 512], F32, tag="Rf", name=f"Rf_{h}_{j}")
                        nc.vector.tensor_add(Rf[:], Rts[0][:], Rts[1][:])
                        rs_ps = stp.tile([1, 512], F32, tag="st", name=f"rs_{h}_{j}")
                        nc.tensor.matmul(
                            rs_ps[:], ones_f[:], Rf[:], start=True, stop=True)
                        recip = misc.tile([1, 512], F32, tag="recip",
                                          name=f"recip_{h}_{j}")
                        nc.vector.reciprocal(recip[:], rs_ps[:])
                        rb = misc.tile([128, 512], F32, tag="rb", name=f"rb_{h}_{j}")
                        nc.gpsimd.partition_broadcast(rb[:], recip[:], channels=128)
                        nc.vector.tensor_mul(
                            ots[h][:, j * 512:(j + 1) * 512], st_h[h]["ot"][:], rb[:])

                # ---- interleaved main loop ----
                # qkv(0) runs alone; during attention(j), fillers emit
                # proj(j-1) then qkv(j+1); leftovers drain between blocks.
                xt_cur = xt3
                xts = {0: xt_cur}
                for u in range(6):
                    for half in range(2):
                        qkv_unit(0, xt_cur, u, half)
                for j in range(NB):
                    if j + 1 < NB:
                        xts[j + 1] = load_x_block(j + 1)
                        nc.sync.dma_start(
                            cosT[:, (j + 1) * 512:(j + 2) * 512],
                            cosT_d[:, (j + 1) * 512:(j + 2) * 512])
                        nc.sync.dma_start(
                            sinT[:, (j + 1) * 512:(j + 2) * 512],
                            sinT_d[:, (j + 1) * 512:(j + 2) * 512])
                        fill_q.append((("qkv", j + 1), qkv_fillers(j + 1, xts[j + 1])))
                    if cfg["phase"] >= 3:
                        drain_until(("qkv", j))
                        if cfg["dual_head"]:
                            attention2(j)
                        else:
                            for h in range(HPC):
                                attention(h, j)
                        if not cfg["interleave"]:
                            drain_fill()
                            if cfg["phase"] >= 4:
                                for f in proj_fillers(j):
                                    f()
                        elif cfg["phase"] >= 4:
                            fill_q.append((("proj", j), proj_fillers(j)))
                            if j == NB - 1:
                                drain_fill()
                    else:
                        drain_fill()
                drain_fill()

                if cfg["phase"] < 4:
                    dummy = misc.tile([128, 16], FP16, tag="dummy", name="dummy")
                    nc.vector.memset(dummy[:], 0.0)
                    nc.sync.dma_start(y_d[0:128, 0:16], dummy[:])

    nc.finalize()
    return nc


def _rope_tables():
    i = np.arange(D)
    denom = np.power(SIN_TIME, 2 * (i // 2) / D)
    pe = np.arange(N)[:, None] / denom[None, :]
    sin = np.sin(pe[:, 0::2])
    cos = np.cos(pe[:, 1::2])
    sin_pos = np.repeat(sin, 2, axis=1)
    cos_pos = np.repeat(cos, 2, axis=1)
    sin_signed = sin_pos.copy()
    sin_signed[:, 0::2] *= -1.0
    perm = np.concatenate([np.arange(0, D, 2), np.arange(1, D, 2)])
    cosT = np.ascontiguousarray(cos_pos.T[perm, :]).astype(np.float16)
    sinT = np.ascontiguousarray(sin_signed.T[perm, :]).astype(np.float16)
    return cosT, sinT, perm


def prep_in_maps(x, W_qkv, W_proj):
    bf = ml_dtypes.bfloat16
    cosT, sinT, perm = _rope_tables()
    xT = np.ascontiguousarray(x.T).astype(bf)
    WpT = W_proj.T
    in_maps = []
    for c in range(NCORES):
        h0, h1 = HPC * c, HPC * c + 1
        blocks = []
        for sec in (0, 1):
            for h in (h0, h1):
                blk = W_qkv[sec * C + h * D: sec * C + (h + 1) * D, :]
                blocks.append(blk[perm, :])
        for h in (h0, h1):
            blocks.append(W_qkv[2 * C + h * D: 2 * C + (h + 1) * D, :])
        shard = np.concatenate(blocks, axis=0)
        wqkvT = np.ascontiguousarray(shard.T).astype(bf)
        wpT = np.ascontiguousarray(WpT[h0 * D:(h1 + 1) * D, :]).astype(bf)
        in_maps.append(
            {"xT": xT, "wqkvT": wqkvT, "wpT": wpT, "cosT": cosT, "sinT": sinT}
        )
    return in_maps


def add_cachetag(in_maps, cfg=None):
    tag = _cache_tag(dict(CFG, **(cfg or {})))
    for m in in_maps:
        m["cachetag"] = np.zeros((tag, 1), np.float32)
    return in_maps


def kernel(x, W_qkv, W_proj, b_proj):
    x = np.asarray(x, dtype=np.float32)
    W_qkv = np.asarray(W_qkv, dtype=np.float32)
    W_proj = np.asarray(W_proj, dtype=np.float32)
    b_proj = np.asarray(b_proj, dtype=np.float32)

    if "nc" not in _CACHE:
        _CACHE["nc"] = build_nc()
    nc = _CACHE["nc"]
    in_maps = add_cachetag(prep_in_maps(x, W_qkv, W_proj))
    res = run_bass_kernel_spmd(nc, in_maps, core_ids=list(range(NCORES)))
    parts = np.stack([res.results[i]["y"] for i in range(NCORES)], axis=0)
    y = parts.sum(axis=0, dtype=np.float32)
    return y + b_proj[None, :]


# revision 5
# speedup vs baseline: 1.1531x; 1.0072x over previous
"""Dev variant: streaming per-block pipeline (QKV(j) -> attention(j) -> proj(j)).

See kernel.py for the algorithm docs. Differences:
  - Single j-loop: per block, QKV projection + rope feed straight into that
    block's attention and output projection, so the PE never drains waiting
    for the ACT exp tail (next block's QKV matmuls are independent).
  - y output in fp16 (partials ~N(0,0.35); fp16 quantization negligible).
  - rowsum on DVE adds by default (frees 288 PE ones-matmuls).
  - final 1/rowsum scale on Pool (keeps DVE hot path short).
"""

import sys

sys.path.insert(0, "/opt/trn_rl_repo")

import numpy as np
import ml_dtypes

import concourse.bass as bass
from concourse import bacc
import concourse.mybir as mybir
import concourse.tile as tile
from concourse.bass_utils import run_bass_kernel_spmd
from concourse.masks import make_identity

N = 4096
C = 2048
H = 16
D = 128
NCORES = 8
HPC = H // NCORES
NB = N // 512
NT = N // 128
CT = C // 128
SCALE = float(D) ** -0.5
SIN_TIME = 10000.0

BF16 = mybir.dt.bfloat16
F32 = mybir.dt.float32
FP16 = mybir.dt.float16

_CACHE = {}


def _cache_tag(cfg):
    import zlib
    with open(__file__, "rb") as f:
        h = zlib.crc32(f.read())
    h = zlib.crc32(repr(sorted(cfg.items())).encode(), h)
    return 16 + (h % 4096)


CFG = dict(
    rowsum_on_pe=False,   # softmax denominator via ones-matmul (else DVE adds)
    racc="fp16",          # rowsum accumulator dtype: "fp16" or "fp32"
    exp_group=2,          # score tiles per exp instruction (1/2/3)
    pipe_depth=7,
    etp_bufs=8,
    r_bufs=2,
    interleave=False,     # feed qkv(j+1)/proj(j-1) PE work into attention gaps
    dual_head=True,       # interleave both heads' score streams in one pipeline
    qswap_psum=False,     # rope pair-swap DMA reads PSUM directly (no ACT copy)
    shared_acc=False,     # all accumulators share mmp ring (needs interleave off)
    fill_per_pair=1,      # filler chunks popped per score-group emitted
    repeat=1,
    phase=4,              # 1=qkv only, 2=+rope/vtrans, 3=+attention, 4=full
)


class _nullpool:
    def __enter__(self):
        return None
    def __exit__(self, *a):
        return False


def build_nc(**overrides):
    cfg = dict(CFG)
    cfg.update(overrides)

    nc = bacc.Bacc(None, target_bir_lowering=False)

    xT_d = nc.dram_tensor("xT", [C, N], BF16, kind="ExternalInput")
    wqkvT_d = nc.dram_tensor("wqkvT", [C, 6 * D], BF16, kind="ExternalInput")
    wpT_d = nc.dram_tensor("wpT", [HPC * D, C], BF16, kind="ExternalInput")
    cosT_d = nc.dram_tensor("cosT", [D, N], FP16, kind="ExternalInput")
    sinT_d = nc.dram_tensor("sinT", [D, N], FP16, kind="ExternalInput")
    y_d = nc.dram_tensor("y", [N, C], FP16, kind="ExternalOutput")
    # compile-cache disambiguator (cache hashes only tensor shapes)
    nc.dram_tensor("cachetag", [_cache_tag(cfg), 1], F32, kind="ExternalInput")

    with tile.TileContext(nc) as tc:
        with (
            tc.tile_pool(name="persist", bufs=1) as persist,
            tc.tile_pool(name="xtp", bufs=2) as xtp,
            tc.tile_pool(name="etp", bufs=cfg["etp_bufs"]) as etp,
            tc.tile_pool(name="ropep", bufs=2) as ropep,
            tc.tile_pool(name="rtabp", bufs=2) as rtabp,
            tc.tile_pool(name="misc", bufs=2) as misc,
            tc.tile_pool(name="ysp", bufs=2) as ysp,
            tc.tile_pool(name="mmp", bufs=2, space="PSUM") as mmp,
            tc.tile_pool(name="otp", bufs=(2 if cfg["dual_head"] else 1),
                         space="PSUM") if not cfg["shared_acc"] else _nullpool() as otp,
            tc.tile_pool(
                name="stp",
                bufs=(max(1, 6 // (2 * cfg["exp_group"])) if cfg["shared_acc"]
                      else max(1, 4 // cfg["exp_group"])),
                space="PSUM") as stp,
            tc.tile_pool(name="rsp", bufs=1, space="PSUM")
            if not (cfg["shared_acc"] or cfg["dual_head"]) else _nullpool() as rsp,
        ):
            import contextlib

            loop_ctx = (
                tc.For_i(0, cfg["repeat"], 1,
                         hint_engines=tuple(nc.engines.keys()))
                if cfg["repeat"] > 1 else contextlib.nullcontext()
            )
            with loop_ctx:
                def load_x_block(j, nchunks=1):
                    t = xtp.tile([128, CT, 512], BF16, tag="xt", name=f"xt_{j}")
                    step = CT // nchunks
                    for s in range(nchunks):
                        nc.sync.dma_start(
                            t[:, s * step:(s + 1) * step, :],
                            xT_d[s * step * 128:(s + 1) * step * 128,
                                 j * 512:(j + 1) * 512].rearrange(
                                "(t p) n -> p t n", p=128
                            ),
                        )
                    return t

                wq_s = []
                for u in range(6):
                    w = persist.tile([128, CT, 128], BF16, tag=f"wq{u}", name=f"wq{u}")
                    wq_s.append(w)

                def load_wq(u):
                    nc.sync.dma_start(
                        wq_s[u][:],
                        wqkvT_d[:, u * D:(u + 1) * D].rearrange("(t p) d -> p t d", p=128),
                    )

                rope_tab = {}

                def load_rope_tab(j):
                    ct = rtabp.tile([128, 512], FP16, tag="cos", name=f"cos_{j}")
                    nc.sync.dma_start(ct[:], cosT_d[:, j * 512:(j + 1) * 512])
                    st = rtabp.tile([128, 512], FP16, tag="sin", name=f"sin_{j}")
                    nc.sync.dma_start(st[:], sinT_d[:, j * 512:(j + 1) * 512])
                    rope_tab[j] = (ct, st)

                load_wq(0)
                xt3 = load_x_block(0, nchunks=4)
                load_rope_tab(0)
                for u in range(1, 6):
                    load_wq(u)
                wp_s = []
                for h in range(HPC):
                    w = persist.tile([128, C], BF16, tag=f"wp{h}", name=f"wp{h}")
                    nc.sync.dma_start(w[:], wpT_d[h * D:(h + 1) * D, :])
                    wp_s.append(w)
                ones = persist.tile([128, 1], BF16, tag="ones", name="ones")
                nc.vector.memset(ones[:], 1.0)
                ones_f = persist.tile([128, 1], F32, tag="ones_f", name="ones_f")
                nc.vector.memset(ones_f[:], 1.0)
                identity = persist.tile([128, 128], BF16, tag="identity", name="identity")
                make_identity(nc, identity[:])
                masks = []
                if True:
                    for mi in range(4):
                        m = persist.tile([128, 512], BF16, tag=f"mask{mi}", name=f"mask{mi}")
                        nc.gpsimd.memset(m[:], 1.0)
                        nc.gpsimd.affine_select(
                            out=m[:], in_=m[:],
                            pattern=[[1, 512]],
                            compare_op=mybir.AluOpType.is_ge,
                            fill=0.0,
                            base=-(mi * 128),
                            channel_multiplier=-1,
                        )
                        masks.append(m)

                qk_store = []
                for u in range(4):
                    t = persist.tile([128, N], BF16, tag=f"qk{u}", name=f"qk{u}")
                    qk_store.append(t)
                v_store = []
                for h in range(HPC):
                    t = persist.tile([128, NT, 128], BF16, tag=f"v{h}", name=f"v{h}")
                    v_store.append(t)
                ots = []
                for h in range(HPC):
                    t = persist.tile([128, N], BF16, tag=f"ot{h}", name=f"ot{h}")
                    ots.append(t)

                def qkv_unit(j, xt3, u, half):
                    """Half a qkv unit: 8 matmuls; second half adds the drain."""
                    if half == 0:
                        ps = mmp.tile([128, 512], F32, tag="mm", name=f"qkvps_{j}_{u}")
                        qkv_ps[(j, u)] = ps
                        for ct in range(CT // 2):
                            nc.tensor.matmul(
                                ps[:], wq_s[u][:, ct, :], xt3[:, ct, :],
                                start=(ct == 0), stop=False,
                            )
                        return
                    ps = qkv_ps.pop((j, u))
                    for ct in range(CT // 2, CT):
                        nc.tensor.matmul(
                            ps[:], wq_s[u][:, ct, :], xt3[:, ct, :],
                            start=False, stop=(ct == CT - 1),
                        )
                    if cfg["phase"] < 2:
                        if u < 4:
                            nc.scalar.copy(qk_store[u][:, j * 512:(j + 1) * 512], ps[:])
                        else:
                            nc.scalar.copy(
                                v_store[u - 4][:, j * 4:(j + 1) * 4, :].rearrange(
                                    "p t n -> p (t n)"), ps[:])
                    elif u < 4:
                        qswap = ropep.tile([128, 512], F32, tag="qswap", name=f"qswap_{j}_{u}")
                        if cfg["qswap_psum"]:
                            nc.sync.dma_start(qswap[0:64, :], ps[64:128, :])
                            nc.sync.dma_start(qswap[64:128, :], ps[0:64, :])
                        else:
                            qraw = ropep.tile([128, 512], F32, tag="qraw", name=f"qraw_{j}_{u}")
                            nc.scalar.copy(qraw[:], ps[:])
                            nc.sync.dma_start(qswap[0:64, :], qraw[64:128, :])
                            nc.sync.dma_start(qswap[64:128, :], qraw[0:64, :])
                        dst = qk_store[u][:, j * 512:(j + 1) * 512]
                        nc.vector.tensor_mul(dst, ps[:], rope_tab[j][0][:])
                        ut = ropep.tile([128, 512], F32, tag="ut", name=f"ut_{j}_{u}")
                        nc.gpsimd.tensor_mul(ut[:], qswap[:], rope_tab[j][1][:])
                        nc.vector.tensor_add(dst, dst, ut[:])
                    else:
                        h = u - 4
                        vtmp = misc.tile([128, 512], BF16, tag="vtmp", name=f"vtmp_{j}_{h}")
                        nc.scalar.copy(vtmp[:], ps[:])
                        for sI in range(4):
                            pst = mmp.tile([128, 128], BF16, tag="mm", name=f"vt_{j}_{h}_{sI}")
                            nc.tensor.transpose(
                                pst[:], vtmp[:, sI * 128:(sI + 1) * 128], identity[:]
                            )
                            nc.vector.tensor_copy(
                                out=v_store[h][:, j * 4 + sI, :], in_=pst[:]
                            )

                qkv_ps = {}

                def qkv_fillers(j, xt3):
                    for u in range(6):
                        for half in range(2):
                            yield lambda u=u, half=half: qkv_unit(j, xt3, u, half)

                def proj_unit(j, nt, half, ci):
                    """One [128,512] slice of the output projection."""
                    key = (nt, half)
                    if ci == 0:
                        ys = ysp.tile([128, C // 2], FP16, tag="ys",
                                      name=f"ys_{nt}_{half}", bufs=4)
                        proj_ys[key] = ys
                    else:
                        ys = proj_ys[key]
                    cc = half * 2 + ci
                    py = mmp.tile([128, 512], F32, tag="mm", name=f"py_{nt}_{cc}")
                    for h in range(HPC):
                        nc.tensor.matmul(
                            py[:], ots[h][:, nt * 128:(nt + 1) * 128],
                            wp_s[h][:, cc * 512:(cc + 1) * 512],
                            start=(h == 0), stop=(h == HPC - 1),
                        )
                    nc.any.tensor_copy(out=ys[:, ci * 512:(ci + 1) * 512], in_=py[:])
                    if ci == 1:
                        del proj_ys[key]
                        nc.sync.dma_start(
                            y_d[nt * 128:(nt + 1) * 128,
                                half * 1024:(half + 1) * 1024], ys[:])

                proj_ys = {}

                def proj_fillers(j):
                    for nt in range(4 * j, 4 * j + 4):
                        for half in range(2):
                            for ci in range(2):
                                yield lambda nt=nt, half=half, ci=ci: proj_unit(j, nt, half, ci)

                from collections import deque
                fill_q = deque()  # entries: [label, generator]

                def pop_fill(k=1):
                    for _ in range(k):
                        if not fill_q:
                            return
                        try:
                            emit = next(fill_q[0][1])
                        except StopIteration:
                            fill_q.popleft()
                            continue
                        emit()

                def drain_fill():
                    while fill_q:
                        pop_fill()

                def drain_until(label):
                    # fully emit the generator carrying `label` (and anything
                    # queued ahead of it)
                    while any(e[0] == label for e in fill_q):
                        pop_fill()

                def attention(h, j):
                    qs = qk_store[h]
                    ks = qk_store[2 + h]
                    ntiles = 4 * j + 4
                    G = cfg["exp_group"]
                    RACC = FP16 if cfg["racc"] == "fp16" else F32
                    acc_pool = mmp if cfg["shared_acc"] else otp
                    ot_ps = acc_pool.tile([128, 512], F32, tag=("mm" if cfg["shared_acc"] else "ot"), name=f"ot_{h}_{j}")
                    if cfg["rowsum_on_pe"]:
                        rs_ps = (mmp if cfg["shared_acc"] else rsp).tile(
                            [1, 512], F32,
                            tag=("mm" if cfg["shared_acc"] else "rs"),
                            name=f"rs_{h}_{j}")
                    else:
                        Rts = []
                        for ri in range(2):
                            Rt = misc.tile([128, 512], RACC, tag=f"R{ri}",
                                           name=f"R{ri}_{h}_{j}", bufs=2)
                            nc.vector.memset(Rt[:], 0.0)
                            Rts.append(Rt)

                    def apply_mask(et_ap, t):
                        if t >= 4 * j:
                            mi = t - 4 * j
                            nc.vector.tensor_mul(et_ap, et_ap, masks[mi][:])

                    def emit_scores_group(tg):
                        ng = min(G, ntiles - tg)
                        stg = stp.tile([128, G, 512], F32, tag="st",
                                       name=f"st_{h}_{j}_{tg}")
                        for i in range(ng):
                            t = tg + i
                            nc.tensor.matmul(
                                stg[:, i, :], ks[:, t * 128:(t + 1) * 128],
                                qs[:, j * 512:(j + 1) * 512],
                                start=True, stop=True,
                            )
                        etg = etp.tile([128, G, 512], BF16, tag="et",
                                       name=f"et_{h}_{j}_{tg}")
                        nc.scalar.activation(
                            etg[:, 0:ng, :], stg[:, 0:ng, :],
                            mybir.ActivationFunctionType.Exp, scale=SCALE,
                        )
                        for i in range(ng):
                            apply_mask(etg[:, i, :], tg + i)
                        return [(tg + i, etg[:, i, :]) for i in range(ng)]

                    def emit_consume(t, et_ap):
                        if cfg["rowsum_on_pe"]:
                            nc.tensor.matmul(
                                rs_ps[:], ones[:], et_ap,
                                start=(t == 0), stop=(t == ntiles - 1),
                                skip_group_check=True,
                            )
                        else:
                            R = Rts[t % 2]
                            nc.vector.tensor_add(R[:], R[:], et_ap)
                        nc.tensor.matmul(
                            ot_ps[:], v_store[h][:, t, :], et_ap,
                            start=(t == 0), stop=(t == ntiles - 1),
                            skip_group_check=True,
                        )

                    depth = cfg["pipe_depth"]
                    pending = deque()
                    nfill = cfg["fill_per_pair"] if cfg["interleave"] else 0
                    for tg in range(0, ntiles, G):
                        pending.append(emit_scores_group(tg))
                        pop_fill(nfill)
                        if len(pending) > depth:
                            for t, ap in pending.popleft():
                                emit_consume(t, ap)
                    while pending:
                        for tt, ap in pending.popleft():
                            emit_consume(tt, ap)

                    if not cfg["rowsum_on_pe"]:
                        Rf = misc.tile([128, 512], F32, tag="Rf", name=f"Rf_{h}_{j}")
                        nc.vector.tensor_add(Rf[:], Rts[0][:], Rts[1][:])
                        rs_pool = mmp if cfg["shared_acc"] else rsp
                        rs_ps = rs_pool.tile([1, 512], F32,
                                             tag=("mm" if cfg["shared_acc"] else "rs"),
                                             name=f"rs_{h}_{j}")
                        nc.tensor.matmul(
                            rs_ps[:], ones_f[:], Rf[:], start=True, stop=True,
                        )
                    recip = misc.tile([1, 512], F32, tag="recip", name=f"recip_{h}_{j}")
                    nc.vector.reciprocal(recip[:], rs_ps[:])
                    rb = misc.tile([128, 512], F32, tag="rb", name=f"rb_{h}_{j}")
                    nc.gpsimd.partition_broadcast(rb[:], recip[:], channels=128)
                    nc.vector.tensor_mul(
                        ots[h][:, j * 512:(j + 1) * 512], ot_ps[:], rb[:]
                    )

                def attention2(j):
                    """Both heads' score streams interleaved in one pipeline so
                    each head's exp/drain tail hides under the other's work."""
                    ntiles = 4 * j + 4
                    G = cfg["exp_group"]
                    RACC = FP16 if cfg["racc"] == "fp16" else F32
                    st_h = {}
                    for h in range(HPC):
                        Rts = []
                        for ri in range(2):
                            Rt = misc.tile([128, 512], RACC, tag=f"R{ri}",
                                           name=f"R{ri}_{h}_{j}",
                                           bufs=cfg["r_bufs"])
                            nc.vector.memset(Rt[:], 0.0)
                            Rts.append(Rt)
                        st_h[h] = dict(
                            ot=otp.tile([128, 512], F32, tag="ot", name=f"ot_{h}_{j}"),
                            R=Rts,
                            qs=qk_store[h], ks=qk_store[2 + h],
                        )

                    def emit_scores_group(h, tg):
                        ng = min(G, ntiles - tg)
                        stg = stp.tile([128, G, 512], F32, tag="st",
                                       name=f"st_{h}_{j}_{tg}")
                        for i in range(ng):
                            t = tg + i
                            nc.tensor.matmul(
                                stg[:, i, :],
                                st_h[h]["ks"][:, t * 128:(t + 1) * 128],
                                st_h[h]["qs"][:, j * 512:(j + 1) * 512],
                                start=True, stop=True,
                            )
                        etg = etp.tile([128, G, 512], BF16, tag="et",
                                       name=f"et_{h}_{j}_{tg}")
                        nc.scalar.activation(
                            etg[:, 0:ng, :], stg[:, 0:ng, :],
                            mybir.ActivationFunctionType.Exp, scale=SCALE,
                        )
                        for i in range(ng):
                            t = tg + i
                            if t >= 4 * j:
                                nc.vector.tensor_mul(
                                    etg[:, i, :], etg[:, i, :], masks[t - 4 * j][:])
                        return [(tg + i, etg[:, i, :]) for i in range(ng)]

                    def emit_consume(h, t, et_ap):
                        nc.vector.tensor_add(
                            st_h[h]["R"][t % 2][:], st_h[h]["R"][t % 2][:], et_ap)
                        nc.tensor.matmul(
                            st_h[h]["ot"][:], v_store[h][:, t, :], et_ap,
                            start=(t == 0), stop=(t == ntiles - 1),
                            skip_group_check=True,
                        )

                    depth = cfg["pipe_depth"]
                    nfill = cfg["fill_per_pair"] if cfg["interleave"] else 0
                    pending = deque()
                    for tg in range(0, ntiles, G):
                        for h in range(HPC):
                            pending.append((h, emit_scores_group(h, tg)))
                            if h == 0:
                                pop_fill(nfill)
                            if len(pending) > depth:
                                hh, items = pending.popleft()
                                for t, ap in items:
                                    emit_consume(hh, t, ap)
                    while pending:
                        hh, items = pending.popleft()
                        for t, ap in items:
                            emit_consume(hh, t, ap)

                    for h in range(HPC):
                        Rts = st_h[h]["R"]
                        Rf = misc.tile([128, 512], F32, tag="Rf", name=f"Rf_{h}_{j}")
                        nc.vector.tensor_add(Rf[:], Rts[0][:], Rts[1][:])
                        rs_ps = stp.tile([1, 512], F32, tag="st", name=f"rs_{h}_{j}")
                        nc.tensor.matmul(
                            rs_ps[:], ones_f[:], Rf[:], start=True, stop=True)
                        recip = misc.tile([1, 512], F32, tag="recip",
                                          name=f"recip_{h}_{j}")
                        nc.vector.reciprocal(recip[:], rs_ps[:])
                        rb = misc.tile([128, 512], F32, tag="rb", name=f"rb_{h}_{j}")
                        nc.gpsimd.partition_broadcast(rb[:], recip[:], channels=128)
                        nc.vector.tensor_mul(
                            ots[h][:, j * 512:(j + 1) * 512], st_h[h]["ot"][:], rb[:])

                # ---- interleaved main loop ----
                # qkv(0) runs alone; during attention(j), fillers emit
                # proj(j-1) then qkv(j+1); leftovers drain between blocks.
                xt_cur = xt3
                xts = {0: xt_cur}
                for u in range(6):
                    for half in range(2):
                        qkv_unit(0, xt_cur, u, half)
                for j in range(NB):
                    if j + 1 < NB:
                        xts[j + 1] = load_x_block(j + 1)
                        load_rope_tab(j + 1)
                        fill_q.append((("qkv", j + 1), qkv_fillers(j + 1, xts[j + 1])))
                    if cfg["phase"] >= 3:
                        drain_until(("qkv", j))
                        if cfg["dual_head"]:
                            attention2(j)
                        else:
                            for h in range(HPC):
                                attention(h, j)
                        if not cfg["interleave"]:
                            drain_fill()
                            if cfg["phase"] >= 4:
                                for f in proj_fillers(j):
                                    f()
                        elif cfg["phase"] >= 4:
                            fill_q.append((("proj", j), proj_fillers(j)))
                            if j == NB - 1:
                                drain_fill()
                    else:
                        drain_fill()
                drain_fill()

                if cfg["phase"] < 4:
                    dummy = misc.tile([128, 16], FP16, tag="dummy", name="dummy")
                    nc.vector.memset(dummy[:], 0.0)
                    nc.sync.dma_start(y_d[0:128, 0:16], dummy[:])

    nc.finalize()
    return nc


def _rope_tables():
    i = np.arange(D)
    denom = np.power(SIN_TIME, 2 * (i // 2) / D)
    pe = np.arange(N)[:, None] / denom[None, :]
    sin = np.sin(pe[:, 0::2])
    cos = np.cos(pe[:, 1::2])
    sin_pos = np.repeat(sin, 2, axis=1)
    cos_pos = np.repeat(cos, 2, axis=1)
    sin_signed = sin_pos.copy()
    sin_signed[:, 0::2] *= -1.0
    perm = np.concatenate([np.arange(0, D, 2), np.arange(1, D, 2)])
    cosT = np.ascontiguousarray(cos_pos.T[perm, :]).astype(np.float16)
    sinT = np.ascontiguousarray(sin_signed.T[perm, :]).astype(np.float16)
    return cosT, sinT, perm


def prep_in_maps(x, W_qkv, W_proj):
    bf = ml_dtypes.bfloat16
    cosT, sinT, perm = _rope_tables()
    xT = np.ascontiguousarray(x.T).astype(bf)
    WpT = W_proj.T
    in_maps = []
    for c in range(NCORES):
        h0, h1 = HPC * c, HPC * c + 1
        blocks = []
        for sec in (0, 1):
            for h in (h0, h1):
                blk = W_qkv[sec * C + h * D: sec * C + (h + 1) * D, :]
                blocks.append(blk[perm, :])
        for h in (h0, h1):
            blocks.append(W_qkv[2 * C + h * D: 2 * C + (h + 1) * D, :])
        shard = np.concatenate(blocks, axis=0)
        wqkvT = np.ascontiguousarray(shard.T).astype(bf)
        wpT = np.ascontiguousarray(WpT[h0 * D:(h1 + 1) * D, :]).astype(bf)
        in_maps.append(
            {"xT": xT, "wqkvT": wqkvT, "wpT": wpT, "cosT": cosT, "sinT": sinT}
        )
    return in_maps


def add_cachetag(in_maps, cfg=None):
    tag = _cache_tag(dict(CFG, **(cfg or {})))
    for m in in_maps:
        m["cachetag"] = np.zeros((tag, 1), np.float32)
    return in_maps


def kernel(x, W_qkv, W_proj, b_proj):
    x = np.asarray(x, dtype=np.float32)
    W_qkv = np.asarray(W_qkv, dtype=np.float32)
    W_proj = np.asarray(W_proj, dtype=np.float32)
    b_proj = np.asarray(b_proj, dtype=np.float32)

    if "nc" not in _CACHE:
        _CACHE["nc"] = build_nc()
    nc = _CACHE["nc"]
    in_maps = add_cachetag(prep_in_maps(x, W_qkv, W_proj))
    res = run_bass_kernel_spmd(nc, in_maps, core_ids=list(range(NCORES)))
    parts = np.stack([res.results[i]["y"] for i in range(NCORES)], axis=0)
    y = parts.sum(axis=0, dtype=np.float32)
    return y + b_proj[None, :]
